# revision 13
# baseline (speedup 1.0000x reference)
# ColBERT MaxSim kernel for Trainium2 (Bass/Tile), 8-core data-parallel.
#
# reference:
#   scores = einsum('bqd,bld->bql', q, d); out = scores.max(-1).sum(-1)   [B]
# shapes: q [64, 32, 128] f32, d [64, 4096, 128] f32  ->  out [64] f32
#
# Sharding: pure data parallel over B — core k handles batches [8k, 8k+8).
#
# Per-core dataflow (per batch b):
#   1. DMA doc[b] [4096,128] -> SBUF X [128, 4096] where partition p holds
#      tokens 32p..32p+31 (one contiguous 16KB read per partition).
#   2. DVE stream-transpose X -> BT (each 32x32 [partition x free] block
#      transposed in place).  Then BT[32r+i, 128s+32c+w] = doc[32(32r+w)+s, 32c+i]:
#      for a fixed (r, c), partitions 32r..32r+31 hold doc dims 32c..32c+31 —
#      the contraction dim lands on partitions, K=32 per block.
#   3. Query q[b] [32,128] is replicated 4x across partition groups and
#      stream-transposed, so block (r, c) of QT is exactly the stationary
#      operand lhsT[i, m] = q[m, 32c+i] at partitions 32r..32r+31.
#   4. 32 matmuls per batch: out[32q x 512tok] psum tiles (r, h), lhsT = QT
#      block (r,c), rhs = BT slice (r, c, h) with 3D AP, accumulating over
#      c = 0..3 (start/stop).  4 row groups run in distinct PE row strips.
#      4 batches of a "quad" write the same psum tiles at partition group
#      32g..32g+32, so psum holds [128, 512] fully packed.
#   5. After each quad: DVE reduce_max per psum bank -> mx [128, 8] ->
#      reduce_max -> qmax[:, quad].  (max over tokens, any order)
#   6. End: stream-transpose qmax [128,32] -> per-batch q-maxes on one
#      partition row; reduce_sum over free -> fin [128, 1]; DMA out [128].
#      Host picks score(local b) = out128[32*(b%4) + b//4].

import numpy as np

B, Q, LD, D = 64, 32, 4096, 128
NCORES = 8
BL = B // NCORES   # 8 local batches per core
NQUAD = BL // 4    # 2 quads of 4 batches

# matmul input dtype: "float32" (exact, 4 cyc/row on PE) or "bfloat16"
# (1 cyc/row; inputs cast f32->bf16 inside the load DMA via SWDGE).
# float32r is unusable: no DVE-transpose support, matmul dst base must be 0.
MM_DTYPE = "bfloat16"

_CACHE = {}


def _build():
    key = MM_DTYPE
    if key in _CACHE:
        return _CACHE[key]

    import concourse.bacc as bacc
    import concourse.bass as bass
    import concourse.tile as tile
    from concourse import mybir

    f32 = mybir.dt.float32
    mmdt = getattr(mybir.dt, MM_DTYPE)
    AX = mybir.AxisListType.X

    cast = MM_DTYPE != "float32"

    nc = bacc.Bacc("TRN2", target_bir_lowering=False, debug=False)
    q_dram = nc.dram_tensor("q_shard", [BL, Q, D], f32, kind="ExternalInput")
    d_dram = nc.dram_tensor("d_shard", [BL, LD, D], f32, kind="ExternalInput")
    out_dram = nc.dram_tensor("out_shard", [128], f32, kind="ExternalOutput")

    with tile.TileContext(nc) as tc:
        with (
            tc.tile_pool(name="xp", bufs=3) as xp,
            tc.tile_pool(name="btp", bufs=2) as btp,
            tc.tile_pool(name="qp", bufs=2) as qp,
            tc.tile_pool(name="mxp", bufs=2) as mxp,
            tc.tile_pool(name="small", bufs=1) as small,
            tc.tile_pool(name="ps", bufs=8, space="PSUM") as ps,
        ):
            qmax = small.tile([128, 32], f32, tag="qmax")
            nc.vector.memset(qmax[:], 0.0)

            for qd in range(NQUAD):
                pts = [
                    ps.tile([128, 512], f32, tag="pt", name=f"pt_{qd}_{i}")
                    for i in range(8)
                ]
                for g in range(4):
                    b = 4 * qd + g
                    # 1. doc load: partition p holds tokens 32p..32p+31.
                    # SWDGE (gpsimd) casts f32->bf16 inline; HWDGE otherwise.
                    X = xp.tile([128, LD], mmdt, tag="x")
                    d_src = d_dram.ap()[b].rearrange("(p s) d -> p (s d)", p=128)
                    if cast:
                        nc.gpsimd.dma_start(out=X[:], in_=d_src)
                    else:
                        nc.sync.dma_start(out=X[:], in_=d_src)
                    # 2. 32x32 block transpose (output rounded to the matmul
                    # input dtype — required for float32r)
                    bt = btp.tile([128, LD], mmdt, tag="bt")
                    nc.vector.transpose(out=bt[:], in_=X[:])

                    # 3. query: replicate across 4 partition groups, block-T
                    qrep = qp.tile([128, D], mmdt, tag="qr")
                    qb = q_dram.ap()[b]
                    qdma = nc.gpsimd.dma_start if cast else nc.sync.dma_start
                    for r in range(4):
                        qdma(out=qrep[32 * r:32 * r + 32, :], in_=qb)
                    qtt = qp.tile([128, D], mmdt, tag="qt")
                    nc.vector.transpose(out=qtt[:], in_=qrep[:])

                    # 4. matmuls: accumulate over c; r spans PE row strips
                    btv = bt[:].rearrange("p (s c w) -> p s c w", c=4, w=32)
                    for h in range(2):
                        for c in range(4):
                            for r in range(4):
                                lhsT = qtt[32 * r:32 * r + 32, 32 * c:32 * c + 32]
                                rhs = btv[32 * r:32 * r + 32, 16 * h:16 * h + 16, c, :]
                                nc.tensor.matmul(
                                    out=pts[r * 2 + h][32 * g:32 * g + 32, :],
                                    lhsT=lhsT,
                                    rhs=rhs,
                                    start=(c == 0),
                                    stop=(c == 3),
                                    tile_position=(32 * r, 32 * g),
                                )

                # 5. quad epilogue: max over tokens at full partition width
                mx = mxp.tile([128, 8], f32, tag="mx")
                for k in range(8):
                    nc.vector.reduce_max(out=mx[:, k:k + 1], in_=pts[k][:], axis=AX)
                nc.vector.reduce_max(out=qmax[:, qd:qd + 1], in_=mx[:], axis=AX)

            # 6. final: transpose so each batch's 32 q-maxes share a partition
            qmaxT = small.tile([128, 32], f32, tag="qmaxT")
            nc.vector.transpose(out=qmaxT[:], in_=qmax[:])
            fin = small.tile([128, 1], f32, tag="fin")
            nc.vector.reduce_sum(out=fin[:], in_=qmaxT[:], axis=AX)
            nc.sync.dma_start(
                out=out_dram.ap().rearrange("(p x) -> p x", x=1), in_=fin[:]
            )

    nc.compile()
    _CACHE[key] = nc
    return nc


def kernel(query_embeds: np.ndarray, doc_embeds: np.ndarray) -> np.ndarray:
    query_embeds = np.ascontiguousarray(np.asarray(query_embeds, dtype=np.float32))
    doc_embeds = np.ascontiguousarray(np.asarray(doc_embeds, dtype=np.float32))
    assert query_embeds.shape == (B, Q, D), query_embeds.shape
    assert doc_embeds.shape == (B, LD, D), doc_embeds.shape

    from concourse.bass_utils import run_bass_kernel_spmd

    nc = _build()

    in_maps = []
    for k in range(NCORES):
        sl = slice(BL * k, BL * (k + 1))
        in_maps.append({
            "q_shard": np.ascontiguousarray(query_embeds[sl]),
            "d_shard": np.ascontiguousarray(doc_embeds[sl]),
        })

    res = run_bass_kernel_spmd(nc, in_maps, core_ids=list(range(NCORES)))

    out = np.empty((B,), dtype=np.float32)
    for k in range(NCORES):
        o = res.results[k]["out_shard"]
        for lb in range(BL):
            out[BL * k + lb] = o[32 * (lb % 4) + lb // 4]
    return out


if __name__ == "__main__":
    rng = np.random.default_rng(0)
    qe = rng.standard_normal((B, Q, D), dtype=np.float32)
    de = rng.standard_normal((B, LD, D), dtype=np.float32)
    got = kernel(qe, de)
    want = (qe @ de.transpose(0, 2, 1)).max(-1).sum(-1)
    err = np.abs(got - want) / np.maximum(1e-6, np.abs(want))
    print("max rel err:", err.max())


# revision 15
# speedup vs baseline: 994.6411x; 994.6411x over previous
# ColBERT MaxSim kernel for Trainium2 (Bass/Tile), 8-core data-parallel.
#
# reference:
#   scores = einsum('bqd,bld->bql', q, d); out = scores.max(-1).sum(-1)   [B]
# shapes: q [64, 32, 128] f32, d [64, 4096, 128] f32  ->  out [64] f32
#
# Sharding: pure data parallel over B — core k handles batches [8k, 8k+8).
#
# Per-core dataflow (per batch b):
#   1. DMA doc[b] [4096,128] -> SBUF X [128, 4096] where partition p holds
#      tokens 32p..32p+31 (one contiguous 16KB read per partition).
#   2. DVE stream-transpose X -> BT (each 32x32 [partition x free] block
#      transposed in place).  Then BT[32r+i, 128s+32c+w] = doc[32(32r+w)+s, 32c+i]:
#      for a fixed (r, c), partitions 32r..32r+31 hold doc dims 32c..32c+31 —
#      the contraction dim lands on partitions, K=32 per block.
#   3. Query q[b] [32,128] is replicated 4x across partition groups and
#      stream-transposed, so block (r, c) of QT is exactly the stationary
#      operand lhsT[i, m] = q[m, 32c+i] at partitions 32r..32r+31.
#   4. 32 matmuls per batch: out[32q x 512tok] psum tiles (r, h), lhsT = QT
#      block (r,c), rhs = BT slice (r, c, h) with 3D AP, accumulating over
#      c = 0..3 (start/stop).  4 row groups run in distinct PE row strips.
#      4 batches of a "quad" write the same psum tiles at partition group
#      32g..32g+32, so psum holds [128, 512] fully packed.
#   5. After each quad: DVE reduce_max per psum bank -> mx [128, 8] ->
#      reduce_max -> qmax[:, quad].  (max over tokens, any order)
#   6. End: stream-transpose qmax [128,32] -> per-batch q-maxes on one
#      partition row; reduce_sum over free -> fin [128, 1]; DMA out [128].
#      Host picks score(local b) = out128[32*(b%4) + b//4].

import numpy as np

B, Q, LD, D = 64, 32, 4096, 128
NCORES = 8
BL = B // NCORES   # 8 local batches per core
NQUAD = BL // 4    # 2 quads of 4 batches

# matmul input dtype: "float32" (exact, 4 cyc/row on PE) or "bfloat16"
# (1 cyc/row; inputs cast f32->bf16 inside the load DMA via SWDGE).
# float32r is unusable: no DVE-transpose support, matmul dst base must be 0.
MM_DTYPE = "bfloat16"

_CACHE = {}


def _build(loop_n=None):
    # loop_n: when set, wrap the whole body in a HW For_i loop (benchmark
    # variant — used by test.py to measure per-iteration HW time by slope).
    key = (MM_DTYPE, loop_n)
    if key in _CACHE:
        return _CACHE[key]

    import contextlib

    import concourse.bacc as bacc
    import concourse.bass as bass
    import concourse.tile as tile
    from concourse import mybir

    f32 = mybir.dt.float32
    mmdt = getattr(mybir.dt, MM_DTYPE)
    AX = mybir.AxisListType.X

    cast = MM_DTYPE != "float32"

    nc = bacc.Bacc("TRN2", target_bir_lowering=False, debug=False)
    q_dram = nc.dram_tensor("q_shard", [BL, Q, D], f32, kind="ExternalInput")
    d_dram = nc.dram_tensor("d_shard", [BL, LD, D], f32, kind="ExternalInput")
    out_dram = nc.dram_tensor("out_shard", [128], f32, kind="ExternalOutput")

    with tile.TileContext(nc) as tc:
        with (
            tc.tile_pool(name="xp", bufs=3) as xp,
            tc.tile_pool(name="btp", bufs=2) as btp,
            tc.tile_pool(name="qp", bufs=2) as qp,
            tc.tile_pool(name="mxp", bufs=2) as mxp,
            tc.tile_pool(name="small", bufs=1) as small,
            tc.tile_pool(name="ps", bufs=8, space="PSUM") as ps,
            tc.For_i(0, loop_n, 1) if loop_n else contextlib.nullcontext(),
        ):
            qmax = small.tile([128, 32], f32, tag="qmax")
            nc.vector.memset(qmax[:], 0.0)

            for qd in range(NQUAD):
                pts = [
                    ps.tile([128, 512], f32, tag="pt", name=f"pt_{qd}_{i}")
                    for i in range(8)
                ]
                for g in range(4):
                    b = 4 * qd + g
                    # 1. doc load: partition p holds tokens 32p..32p+31.
                    # SWDGE (gpsimd) casts f32->bf16 inline; HWDGE otherwise.
                    X = xp.tile([128, LD], mmdt, tag="x")
                    d_src = d_dram.ap()[b].rearrange("(p s) d -> p (s d)", p=128)
                    if cast:
                        nc.gpsimd.dma_start(out=X[:], in_=d_src)
                    else:
                        nc.sync.dma_start(out=X[:], in_=d_src)
                    # 2. 32x32 block transpose (output rounded to the matmul
                    # input dtype — required for float32r)
                    bt = btp.tile([128, LD], mmdt, tag="bt")
                    nc.vector.transpose(out=bt[:], in_=X[:])

                    # 3. query: replicate across 4 partition groups, block-T
                    qrep = qp.tile([128, D], mmdt, tag="qr")
                    qb = q_dram.ap()[b]
                    qdma = nc.gpsimd.dma_start if cast else nc.sync.dma_start
                    for r in range(4):
                        qdma(out=qrep[32 * r:32 * r + 32, :], in_=qb)
                    qtt = qp.tile([128, D], mmdt, tag="qt")
                    nc.vector.transpose(out=qtt[:], in_=qrep[:])

                    # 4. matmuls: accumulate over c; r spans PE row strips
                    btv = bt[:].rearrange("p (s c w) -> p s c w", c=4, w=32)
                    for h in range(2):
                        for c in range(4):
                            for r in range(4):
                                lhsT = qtt[32 * r:32 * r + 32, 32 * c:32 * c + 32]
                                rhs = btv[32 * r:32 * r + 32, 16 * h:16 * h + 16, c, :]
                                nc.tensor.matmul(
                                    out=pts[r * 2 + h][32 * g:32 * g + 32, :],
                                    lhsT=lhsT,
                                    rhs=rhs,
                                    start=(c == 0),
                                    stop=(c == 3),
                                    tile_position=(32 * r, 32 * g),
                                )

                # 5. quad epilogue: max over tokens at full partition width
                mx = mxp.tile([128, 8], f32, tag="mx")
                for k in range(8):
                    nc.vector.reduce_max(out=mx[:, k:k + 1], in_=pts[k][:], axis=AX)
                nc.vector.reduce_max(out=qmax[:, qd:qd + 1], in_=mx[:], axis=AX)

            # 6. final: transpose so each batch's 32 q-maxes share a partition
            qmaxT = small.tile([128, 32], f32, tag="qmaxT")
            nc.vector.transpose(out=qmaxT[:], in_=qmax[:])
            fin = small.tile([128, 1], f32, tag="fin")
            nc.vector.reduce_sum(out=fin[:], in_=qmaxT[:], axis=AX)
            nc.sync.dma_start(
                out=out_dram.ap().rearrange("(p x) -> p x", x=1), in_=fin[:]
            )

    nc.compile()
    _CACHE[key] = nc
    return nc


def kernel(query_embeds: np.ndarray, doc_embeds: np.ndarray) -> np.ndarray:
    query_embeds = np.ascontiguousarray(np.asarray(query_embeds, dtype=np.float32))
    doc_embeds = np.ascontiguousarray(np.asarray(doc_embeds, dtype=np.float32))
    assert query_embeds.shape == (B, Q, D), query_embeds.shape
    assert doc_embeds.shape == (B, LD, D), doc_embeds.shape

    from concourse.bass_utils import run_bass_kernel_spmd

    nc = _build()

    in_maps = []
    for k in range(NCORES):
        sl = slice(BL * k, BL * (k + 1))
        in_maps.append({
            "q_shard": np.ascontiguousarray(query_embeds[sl]),
            "d_shard": np.ascontiguousarray(doc_embeds[sl]),
        })

    res = run_bass_kernel_spmd(nc, in_maps, core_ids=list(range(NCORES)))

    out = np.empty((B,), dtype=np.float32)
    for k in range(NCORES):
        o = res.results[k]["out_shard"]
        for lb in range(BL):
            out[BL * k + lb] = o[32 * (lb % 4) + lb // 4]
    return out


if __name__ == "__main__":
    rng = np.random.default_rng(0)
    qe = rng.standard_normal((B, Q, D), dtype=np.float32)
    de = rng.standard_normal((B, LD, D), dtype=np.float32)
    got = kernel(qe, de)
    want = (qe @ de.transpose(0, 2, 1)).max(-1).sum(-1)
    err = np.abs(got - want) / np.maximum(1e-6, np.abs(want))
    print("max rel err:", err.max())
